# revision 30
# baseline (speedup 1.0000x reference)
"""Multi-head attention (B=8, S=1024, D=768, H=12) on 8 TRN2 NeuronCores.

Strategy: pure data parallelism — core b computes batch element b end-to-end;
weights are replicated. The host pre-transposes x and the weight matrices so
the contraction axis (d) lands on SBUF partitions with no on-device
transposes, and pre-casts matmul operands to bf16 (f32 PSUM accumulation).

Per-core dataflow (PE-queue order is the schedule; engines execute in-order):
  input DMAs issued from SP+ACT+DVE queues in consumption order so the first
    QKV matmuls start ~6us in and the full load overlaps compute
  qk^T  [e,s]  = Wqkv^T(d,e)-blocks.T @ x^T(d,s)       (+bias on DVE)
  v_aug [s,12,65] = x^T-blocks.T @ Wqkv^T(d, v-cols)   (+bias via DVE add of a
                     PE-broadcast bias tile; col 64 of each head block = 1.0
                     for softmax denominators), spread 2 per head over heads 1-4
  per head h (AV lags scores by AV_LAG heads so the tail stays PE-dense):
    scores^T[sk,sq] = k^T-block.T @ q^T     (K=64 contraction)
    attn^T = Exp(scores^T * 1/8)            (ACT; no max-pass: scores ~ N(0,1))
    out[sq,65] = attn^T-block.T @ v_aug     (attn stationary: full M=128 rate;
                  col 64 = softmax denominator -> per-partition normalization)
    projin^T = PE-transpose(out * 1/denom)  (restores [d, s] layout for proj)
  y[s,e] = projin^T-blocks.T @ Wproj^T      (optionally split into a ki 0-3
            partial before the tail AVs + ki 4-5 remainder, so the PE has
            dense work while the last heads' exps drain on ACT)
"""

import sys

sys.path.insert(0, "/opt/trn_rl_repo")

import contextlib

import numpy as np
import ml_dtypes

import concourse.bass as bass
from concourse import bacc, mybir
import concourse.tile as tile
from concourse.masks import make_identity

S = 1024
D = 768
E3 = 3 * D
H = 12
DH = D // H
SCALE = DH ** -0.5
N_CORES = 8

F32 = mybir.dt.float32
F32R = mybir.dt.float32r
BF16 = mybir.dt.bfloat16

# schedule knobs
N_WARM = 8  # PE warm-up matmuls of [128, 512] covering the input-DMA window
V_AT = {1: (0, 2), 2: (2, 4), 3: (4, 6), 4: (6, 8)}  # head -> v_stile range
AV_LAG = 4  # AV(h) is queued after scores(h + AV_LAG)
PROJ_SPLIT = True  # ki 0-3 partial proj before tail AVs, ki 4-5 after


def build_nc(do_compile=True, loop_k=None, with_bias=True):
    nc = bacc.Bacc()

    xT_d = nc.declare_dram_parameter("xT", [D, S], BF16, isOutput=False)
    wqkvT_d = nc.declare_dram_parameter("wqkvT", [D, E3], BF16, isOutput=False)
    wprojT_d = nc.declare_dram_parameter("wprojT", [D, D], BF16, isOutput=False)
    qkvb_d = nc.declare_dram_parameter("qkvb", [E3], F32, isOutput=False)
    projb_d = nc.declare_dram_parameter("projb", [D], F32, isOutput=False)
    # bf16 output: halves the output-DMA bytes; the host converts back to
    # f32 (costs ~0.1% extra rel err against a 2e-2 budget)
    out_d = nc.declare_dram_parameter("out", [S, D], BF16, isOutput=True)

    with tile.TileContext(nc) as tc:
        with (
            tc.For_i(
                0,
                loop_k,
                1,
                hint_engines=(
                    mybir.EngineType.PE,
                    mybir.EngineType.Activation,
                    mybir.EngineType.DVE,
                    mybir.EngineType.SP,
                    mybir.EngineType.Pool,
                ),
            )
            if loop_k
            else contextlib.nullcontext()
        ), tc.tile_pool(name="sb", bufs=1) as sb, tc.tile_pool(
            name="psmm", bufs=2, space="PSUM"
        ) as ps_mm, tc.tile_pool(
            name="pssc", bufs=2, space="PSUM"
        ) as ps_sc, tc.tile_pool(
            name="psx", bufs=2, space="PSUM"
        ) as ps_x:
            # ---- constants ----
            # warm-up operands first: the warm matmuls self-feed from warm_sb
            # (junk output, discarded) so they don't wait on identity
            warm_sb = sb.tile([128, 512], BF16, tag="warm")
            nc.gpsimd.memset(warm_sb, 0.0)
            identity = sb.tile([128, 128], BF16, tag="ident")
            make_identity(nc, identity)
            ident32 = sb.tile([128, 128], F32R, tag="ident32")
            nc.vector.tensor_copy(out=ident32, in_=identity)
            if with_bias:
                ones_row = sb.tile([1, 128], BF16, tag="ones")
                nc.vector.memset(ones_row, 1.0)
                qb_col = sb.tile([128, 12], F32, tag="qbcol")
                nc.sync.dma_start(
                    out=qb_col, in_=qkvb_d[0 : 12 * 128].rearrange("(j p) -> p j", p=128)
                )
                vb_f32 = sb.tile([1, D], F32, tag="vbrow32")
                nc.sync.dma_start(out=vb_f32, in_=qkvb_d[2 * D : 3 * D][None, :])
                vb_row = sb.tile([1, D], BF16, tag="vbrow")
                nc.vector.tensor_copy(out=vb_row, in_=vb_f32)
                pb_f32 = sb.tile([1, D], F32, tag="pbrow32")
                nc.sync.dma_start(out=pb_f32, in_=projb_d[None, :])
                pb_row = sb.tile([1, D], BF16, tag="pbrow")
                nc.vector.tensor_copy(out=pb_row, in_=pb_f32)

            # ---- input DMAs: multi-engine issue, consumption order ----
            # SP: q/k columns for the first etile pair, then the remaining q/k
            #     columns, then wproj. ACT: x^T (its exp stream starts ~10us
            #     in). DVE: v columns (needed by v_stiles from head 1 on).
            xt = [sb.tile([128, S], BF16, tag=f"xt{i}", name=f"xt{i}") for i in range(6)]
            wq = [sb.tile([128, E3], BF16, tag=f"wq{i}", name=f"wq{i}") for i in range(6)]
            wp = [sb.tile([128, D], BF16, tag=f"wp{i}", name=f"wp{i}") for i in range(6)]

            def _qk2(i):
                src = wqkvT_d[128 * i : 128 * (i + 1), 0 : 2 * D].rearrange(
                    "p (b c) -> p b c", b=2
                )
                dst = wq[i][:, 0 : 2 * D].rearrange("p (b c) -> p b c", b=2)
                return src, dst

            for i in range(6):
                src, dst = _qk2(i)
                nc.sync.dma_start(out=dst[:, :, 0:128], in_=src[:, :, 0:128])
            for i in range(6):
                nc.scalar.dma_start(
                    out=xt[i][:, 0:512], in_=xT_d[128 * i : 128 * (i + 1), 0:512]
                )
            for i in range(6):
                nc.gpsimd.dma_start(
                    out=xt[i][:, 512:1024],
                    in_=xT_d[128 * i : 128 * (i + 1), 512:1024],
                )
            for i in range(6):
                nc.gpsimd.dma_start(
                    out=wq[i][:, 2 * D : E3],
                    in_=wqkvT_d[128 * i : 128 * (i + 1), 2 * D : E3],
                )
            for i in range(6):
                src, dst = _qk2(i)
                nc.sync.dma_start(out=dst[:, :, 128:768], in_=src[:, :, 128:768])
            for i in range(6):
                nc.sync.dma_start(
                    out=wp[i], in_=wprojT_d[128 * i : 128 * (i + 1), :]
                )

            # ---- PE clock warm-up ----
            # The HAM clock gate runs the PE at half rate until ~3.4us of
            # sustained activity. Burn the input-DMA wait on throwaway
            # matmuls so the first real matmuls run at full clock.
            for w in range(N_WARM):
                wps = ps_x.tile([128, 512], F32, tag="px", name="pswarm")
                nc.tensor.matmul(
                    wps, warm_sb[:, 0:128], warm_sb, start=True, stop=True
                )

            # ---- bias broadcast tiles (PE ones-matmul; replaces per-stile
            # bias matmuls). Doubles as extra warm-up. ----
            if with_bias:
                vb_bc = sb.tile([128, D], BF16, tag="vbbc")
                pb_bc = sb.tile([128, D], BF16, tag="pbbc")
                for half in range(2):
                    cols = slice(384 * half, 384 * half + 384)
                    ps = ps_mm.tile([128, 384], F32, tag="mm", name="psbc")
                    nc.tensor.matmul(
                        ps, ones_row[:, 0:128], vb_row[:, cols], start=True, stop=True
                    )
                    nc.vector.tensor_copy(out=vb_bc[:, cols], in_=ps)
                    ps2 = ps_mm.tile([128, 384], F32, tag="mm", name="psbc2")
                    nc.tensor.matmul(
                        ps2, ones_row[:, 0:128], pb_row[:, cols], start=True, stop=True
                    )
                    nc.vector.tensor_copy(out=pb_bc[:, cols], in_=ps2)

            # q/k stored as half-tiles so scores can start as soon as the
            # producing QKV half-group drains (finer Tile dep granularity)
            qkTh = [
                [
                    sb.tile([128, 512], BF16, tag=f"qk{i}h{f}", name=f"qk{i}h{f}")
                    for f in range(2)
                ]
                for i in range(12)
            ]
            v_aug = [
                sb.tile([128, H, DH + 1], BF16, tag=f"va{i}", name=f"va{i}")
                for i in range(8)
            ]
            projin = [
                sb.tile([128, S], BF16, tag=f"pj{i}", name=f"pj{i}") for i in range(6)
            ]
            if PROJ_SPLIT:
                ypart = [
                    sb.tile([128, D], F32R, tag=f"yp{i}", name=f"yp{i}")
                    for i in range(8)
                ]

            def qk_etile(et):
                # q/k column block [e, s] with per-e bias, accumulated over d
                for hf in range(2):
                    ps = ps_mm.tile([128, 512], F32, tag="mm", name="psqk")
                    for ki in range(6):
                        nc.tensor.matmul(
                            ps,
                            wq[ki][:, 128 * et : 128 * et + 128],
                            xt[ki][:, 512 * hf : 512 * hf + 512],
                            start=(ki == 0),
                            stop=(ki == 5),
                        )
                    if with_bias:
                        nc.vector.tensor_scalar_add(
                            qkTh[et][hf], ps, qb_col[:, et : et + 1]
                        )
                    else:
                        nc.vector.tensor_copy(out=qkTh[et][hf], in_=ps)

            def v_stile(st):
                nc.vector.memset(v_aug[st][:, :, DH : DH + 1], 1.0)
                for hf in range(2):
                    ps = ps_mm.tile([128, 384], F32, tag="mm", name="psv")
                    for ki in range(6):
                        nc.tensor.matmul(
                            ps,
                            xt[ki][:, 128 * st : 128 * st + 128],
                            wq[ki][:, 2 * D + 384 * hf : 2 * D + 384 * hf + 384],
                            start=(ki == 0),
                            stop=(ki == 5),
                        )
                    dst = v_aug[st][:, 6 * hf : 6 * hf + 6, 0:DH]
                    src = ps.rearrange("p (h d) -> p h d", h=6)
                    if with_bias:
                        nc.vector.tensor_tensor(
                            out=dst,
                            in0=src,
                            in1=vb_bc[:, 384 * hf : 384 * hf + 384].rearrange(
                                "p (h d) -> p h d", h=6
                            ),
                            op=mybir.AluOpType.add,
                        )
                    else:
                        nc.vector.tensor_copy(out=dst, in_=src)

            def head_scores(h):
                row = 64 * (h % 2)

                at = []
                for sk in range(8):
                    k_half = qkTh[6 + h // 2][sk // 4]
                    k_blk = k_half[row : row + 64, 128 * (sk % 4) : 128 * (sk % 4) + 128]
                    ps = ps_sc.tile([128, S], F32, tag="sc", name="pssc")
                    for hf in range(2):
                        nc.tensor.matmul(
                            ps[:, 512 * hf : 512 * hf + 512],
                            k_blk,
                            qkTh[h // 2][hf][row : row + 64, :],
                            start=True,
                            stop=True,
                        )
                    a = sb.tile([128, S], BF16, tag="at", bufs=32, name="at")
                    nc.scalar.activation(
                        out=a,
                        in_=ps,
                        func=mybir.ActivationFunctionType.Exp,
                        scale=SCALE,
                    )
                    at.append(a)
                return at

            def head_av_nat(h, at, g):
                # AV with attn^T stationary: out natural [sq, 65], M=128 rate.
                # 4 sq-blocks batched per PSUM bank to amortize engine hops;
                # normalize per-partition (sq). Returns the scaled tile for
                # the deferred transpose pass.
                nat = ps_x.tile([128, 4 * (DH + 1)], F32, tag="px", name="psnat")
                nat_r = nat.rearrange("p (j c) -> p j c", c=DH + 1)
                for sk in range(8):
                    for j in range(4):
                        nc.tensor.matmul(
                            nat_r[:, j, :],
                            at[sk][:, 128 * (4 * g + j) : 128 * (4 * g + j) + 128],
                            v_aug[sk][:, h, :],
                            start=(sk == 0 and j == 0),
                            stop=(sk == 7 and j == 3),
                        )
                rec = sb.tile([128, 4], F32, tag="rec", bufs=3, name="rec")
                nc.vector.reciprocal(out=rec, in_=nat_r[:, :, DH])
                scaled = sb.tile([128, 4 * DH], BF16, tag="scaled", bufs=3, name="scaled")
                rec_b = bass.AP(
                    tensor=rec.tensor,
                    offset=rec.offset,
                    ap=[rec.ap[0], rec.ap[1], [0, DH]],
                )
                nc.vector.tensor_mul(
                    scaled.rearrange("p (j d) -> p j d", j=4),
                    nat_r[:, :, 0:DH],
                    rec_b,
                )
                return scaled

            def head_av_tp(h, g, scaled):
                # PE-transpose the normalized AV block back to [d, sq]
                for t in range(2):
                    tp = ps_x.tile([128, 128], BF16, tag="px", name="pstp")
                    nc.tensor.transpose(
                        tp, scaled[:, 128 * t : 128 * t + 128], identity
                    )
                    for u in range(2):
                        sq = 4 * g + 2 * t + u
                        nc.vector.tensor_copy(
                            out=projin[h // 2][
                                64 * (h % 2) : 64 * (h % 2) + 64,
                                128 * sq : 128 * sq + 128,
                            ],
                            in_=tp[64 * u : 64 * u + 64, :],
                        )

            def head_av(h, at):
                for g in range(2):
                    scaled = head_av_nat(h, at, g)
                    head_av_tp(h, g, scaled)

            def proj_hf(st, kis, hf, first, last, split_pools, y):
                # one hf-half ki-range partial of the projection for block st.
                # Non-first groups fold the f32 partial back in on the PE via
                # an f32r identity matmul (1 cycle/row), so the drain is a
                # plain PSUM->SBUF copy that ACT and DVE split between them.
                pl = ps_sc if (split_pools and hf == 1) else ps_mm
                ps = pl.tile([128, 384], F32, tag="mm" if pl is ps_mm else "sc", name="psy")
                for n, ki in enumerate(kis):
                    nc.tensor.matmul(
                        ps,
                        projin[ki][:, 128 * st : 128 * st + 128],
                        wp[ki][:, 384 * hf : 384 * hf + 384],
                        start=(n == 0),
                        stop=(first and n == len(kis) - 1),
                    )
                cols = slice(384 * hf, 384 * hf + 384)
                if not first:
                    nc.tensor.matmul(
                        ps,
                        ident32,
                        ypart[st][:, cols],
                        start=False,
                        stop=True,
                    )
                if last:
                    if with_bias and first:
                        nc.vector.tensor_tensor(
                            out=y[:, cols], in0=ps, in1=pb_bc[:, cols],
                            op=mybir.AluOpType.add,
                        )
                    elif hf == 0:
                        nc.scalar.activation(
                            out=y[:, cols], in_=ps,
                            func=mybir.ActivationFunctionType.Copy,
                        )
                    else:
                        nc.vector.tensor_copy(out=y[:, cols], in_=ps)
                else:
                    if with_bias:
                        nc.vector.tensor_tensor(
                            out=ypart[st][:, cols], in0=ps, in1=pb_bc[:, cols],
                            op=mybir.AluOpType.add,
                        )
                    else:
                        nc.vector.tensor_copy(out=ypart[st][:, cols], in_=ps)

            def proj_group(st, kis, first, last, split_pools=False, dma_eng=None):
                y = (
                    sb.tile([128, D], BF16, tag="y", bufs=3, name="y")
                    if last
                    else None
                )
                for hf in range(2):
                    proj_hf(st, kis, hf, first, last, split_pools, y)
                if last:
                    eng = dma_eng if dma_eng is not None else nc.sync
                    eng.dma_start(out=out_d[128 * st : 128 * st + 128, :], in_=y)

            # ---- interleaved schedule ----
            ats = {}
            pend = []
            for p in range(6):
                qk_etile(p)
                qk_etile(6 + p)
                for h in (2 * p, 2 * p + 1):
                    if h in V_AT:
                        for st in range(*V_AT[h]):
                            v_stile(st)
                    ats[h] = head_scores(h)
                    pend.append(h)
                    if len(pend) > AV_LAG:
                        hh = pend.pop(0)
                        head_av(hh, ats.pop(hh))

            # tail phase: scores psum pool is free from here — alternate proj
            # groups across ps_mm/ps_sc so PSUM drains overlap, and weave the
            # remaining AVs between proj pairs to cover drain latency.
            if PROJ_SPLIT:
                # AV nat groups lead (their exps are long done); each proj-A
                # group covers the normalize->transpose latency of the AV
                # half it is woven between. proj-B + out-DMA stream last so
                # the DMA engines start flowing as early as possible.
                avq = [(h, g) for h in pend for g in range(2)]
                for st in range(8):
                    h, g = avq[st]
                    scaled = head_av_nat(h, ats[h], g)
                    proj_hf(st, range(4), 0, True, False, True, None)
                    head_av_tp(h, g, scaled)
                    proj_hf(st, range(4), 1, True, False, True, None)
                for hh in pend:
                    ats.pop(hh)
                for st in range(8):
                    proj_group(
                        st, range(4, 6), first=False, last=True,
                        split_pools=True,
                        dma_eng=nc.sync if st % 2 == 0 else nc.scalar,
                    )
            else:
                for hh in pend:
                    head_av(hh, ats.pop(hh))
                for st in range(8):
                    proj_group(
                        st, range(6), first=True, last=True,
                        split_pools=True,
                        dma_eng=nc.sync if st % 2 == 0 else nc.scalar,
                    )

    if do_compile:
        nc.compile()
    return nc


_NCS = {}


def _get_nc(with_bias=True):
    if with_bias not in _NCS:
        _NCS[with_bias] = build_nc(with_bias=with_bias)
    return _NCS[with_bias]


def make_in_maps(x, qkv_w, qkv_b, proj_w, proj_b):
    x = np.asarray(x, dtype=np.float32)
    qkv_w = np.asarray(qkv_w, dtype=np.float32)
    qkv_b = np.asarray(qkv_b, dtype=np.float32)
    proj_w = np.asarray(proj_w, dtype=np.float32)
    proj_b = np.asarray(proj_b, dtype=np.float32)

    xT = np.ascontiguousarray(x.transpose(0, 2, 1)).astype(ml_dtypes.bfloat16)
    wqkvT = np.ascontiguousarray(qkv_w.T).astype(ml_dtypes.bfloat16)
    wprojT = np.ascontiguousarray(proj_w.T).astype(ml_dtypes.bfloat16)
    return [
        {
            "xT": xT[b],
            "wqkvT": wqkvT,
            "wprojT": wprojT,
            "qkvb": qkv_b,
            "projb": proj_b,
        }
        for b in range(N_CORES)
    ]


def kernel(x, qkv_w, qkv_b, proj_w, proj_b):
    from concourse.bass_utils import run_bass_kernel_spmd

    in_maps = make_in_maps(x, qkv_w, qkv_b, proj_w, proj_b)
    with_bias = bool(np.any(np.asarray(qkv_b)) or np.any(np.asarray(proj_b)))
    nc = _get_nc(with_bias)
    res = run_bass_kernel_spmd(nc, in_maps, core_ids=list(range(N_CORES))).results
    return np.stack([res[b]["out"] for b in range(N_CORES)]).astype(np.float32)


# revision 37
# speedup vs baseline: 2.9404x; 2.9404x over previous
"""Multi-head attention (B=8, S=1024, D=768, H=12) on 8 TRN2 NeuronCores.

Strategy: pure data parallelism — core b computes batch element b end-to-end;
weights are replicated. The host pre-transposes x and the weight matrices so
the contraction axis (d) lands on SBUF partitions with no on-device
transposes, and pre-casts matmul operands to bf16 (f32 PSUM accumulation).

Per-core dataflow (PE-queue order is the schedule; engines execute in-order):
  input DMAs issued from SP+ACT+DVE queues in consumption order so the first
    QKV matmuls start ~6us in and the full load overlaps compute
  qk^T  [e,s]  = Wqkv^T(d,e)-blocks.T @ x^T(d,s)       (+bias on DVE)
  v_aug [s,12,65] = x^T-blocks.T @ Wqkv^T(d, v-cols)   (+bias via DVE add of a
                     PE-broadcast bias tile; col 64 of each head block = 1.0
                     for softmax denominators), spread 2 per head over heads 1-4
  per head h (AV lags scores by AV_LAG heads so the tail stays PE-dense):
    scores^T[sk,sq] = k^T-block.T @ q^T     (K=64 contraction)
    attn^T = Exp(scores^T * 1/8)            (ACT; no max-pass: scores ~ N(0,1))
    out[sq,65] = attn^T-block.T @ v_aug     (attn stationary: full M=128 rate;
                  col 64 = softmax denominator -> per-partition normalization)
    projin^T = PE-transpose(out * 1/denom)  (restores [d, s] layout for proj)
  y[s,e] = projin^T-blocks.T @ Wproj^T      (optionally split into a ki 0-3
            partial before the tail AVs + ki 4-5 remainder, so the PE has
            dense work while the last heads' exps drain on ACT)
"""

import sys

sys.path.insert(0, "/opt/trn_rl_repo")

import contextlib

import numpy as np
import ml_dtypes

import concourse.bass as bass
from concourse import bacc, mybir
import concourse.tile as tile
from concourse.masks import make_identity

S = 1024
D = 768
E3 = 3 * D
H = 12
DH = D // H
SCALE = DH ** -0.5
N_CORES = 8

F32 = mybir.dt.float32
F32R = mybir.dt.float32r
BF16 = mybir.dt.bfloat16

# schedule knobs
N_WARM = 8  # PE warm-up matmuls of [128, 512] covering the input-DMA window
V_AT = {1: (0, 2), 2: (2, 4), 3: (4, 6), 4: (6, 8)}  # head -> v_stile range
AV_LAG = 4  # AV(h) is queued after scores(h + AV_LAG)
PROJ_SPLIT = False  # ki 0-3 partial proj before tail AVs, ki 4-5 after
DMA_PLAN = "sp_act"  # contiguous chunks; x^T on ACT queue, rest on SP


def build_nc(
    do_compile=True,
    loop_k=None,
    with_bias=True,
    dma_plan=None,  # None -> module default DMA_PLAN
    proj_split=None,  # None -> module default PROJ_SPLIT
):
    if dma_plan is None:
        dma_plan = DMA_PLAN
    if proj_split is None:
        proj_split = PROJ_SPLIT
    nc = bacc.Bacc()

    xT_d = nc.declare_dram_parameter("xT", [D, S], BF16, isOutput=False)
    wqkvT_d = nc.declare_dram_parameter("wqkvT", [D, E3], BF16, isOutput=False)
    wprojT_d = nc.declare_dram_parameter("wprojT", [D, D], BF16, isOutput=False)
    qkvb_d = nc.declare_dram_parameter("qkvb", [E3], F32, isOutput=False)
    projb_d = nc.declare_dram_parameter("projb", [D], F32, isOutput=False)
    # bf16 output: halves the output-DMA bytes; the host converts back to
    # f32 (costs ~0.1% extra rel err against a 2e-2 budget)
    out_d = nc.declare_dram_parameter("out", [S, D], BF16, isOutput=True)

    with tile.TileContext(nc) as tc:
        with (
            tc.For_i(
                0,
                loop_k,
                1,
                hint_engines=(
                    mybir.EngineType.PE,
                    mybir.EngineType.Activation,
                    mybir.EngineType.DVE,
                    mybir.EngineType.SP,
                    mybir.EngineType.Pool,
                ),
            )
            if loop_k
            else contextlib.nullcontext()
        ), tc.tile_pool(name="sb", bufs=1) as sb, tc.tile_pool(
            name="psmm", bufs=2, space="PSUM"
        ) as ps_mm, tc.tile_pool(
            name="pssc", bufs=2, space="PSUM"
        ) as ps_sc, tc.tile_pool(
            name="psx", bufs=2, space="PSUM"
        ) as ps_x:
            # ---- constants ----
            # warm-up operands first: the warm matmuls self-feed from warm_sb
            # (junk output, discarded) so they don't wait on identity
            warm_sb = sb.tile([128, 512], BF16, tag="warm")
            nc.gpsimd.memset(warm_sb, 0.0)
            identity = sb.tile([128, 128], BF16, tag="ident")
            make_identity(nc, identity)
            ident32 = sb.tile([128, 128], F32R, tag="ident32")
            nc.vector.tensor_copy(out=ident32, in_=identity)
            if with_bias:
                ones_row = sb.tile([1, 128], BF16, tag="ones")
                nc.vector.memset(ones_row, 1.0)
                qb_col = sb.tile([128, 12], F32, tag="qbcol")
                nc.sync.dma_start(
                    out=qb_col, in_=qkvb_d[0 : 12 * 128].rearrange("(j p) -> p j", p=128)
                )
                vb_f32 = sb.tile([1, D], F32, tag="vbrow32")
                nc.sync.dma_start(out=vb_f32, in_=qkvb_d[2 * D : 3 * D][None, :])
                vb_row = sb.tile([1, D], BF16, tag="vbrow")
                nc.vector.tensor_copy(out=vb_row, in_=vb_f32)
                pb_f32 = sb.tile([1, D], F32, tag="pbrow32")
                nc.sync.dma_start(out=pb_f32, in_=projb_d[None, :])
                pb_row = sb.tile([1, D], BF16, tag="pbrow")
                nc.vector.tensor_copy(out=pb_row, in_=pb_f32)

            # ---- input DMAs: multi-engine issue, consumption order ----
            # SP: q/k columns for the first etile pair, then the remaining q/k
            #     columns, then wproj. ACT: x^T (its exp stream starts ~10us
            #     in). DVE: v columns (needed by v_stiles from head 1 on).
            xt = [sb.tile([128, S], BF16, tag=f"xt{i}", name=f"xt{i}") for i in range(6)]
            wq = [sb.tile([128, E3], BF16, tag=f"wq{i}", name=f"wq{i}") for i in range(6)]
            wp = [sb.tile([128, D], BF16, tag=f"wp{i}", name=f"wp{i}") for i in range(6)]

            if dma_plan == "multi":
                def _qk2(i):
                    src = wqkvT_d[128 * i : 128 * (i + 1), 0 : 2 * D].rearrange(
                        "p (b c) -> p b c", b=2
                    )
                    dst = wq[i][:, 0 : 2 * D].rearrange("p (b c) -> p b c", b=2)
                    return src, dst

                for i in range(6):
                    src, dst = _qk2(i)
                    nc.sync.dma_start(out=dst[:, :, 0:128], in_=src[:, :, 0:128])
                for i in range(6):
                    nc.scalar.dma_start(
                        out=xt[i][:, 0:512], in_=xT_d[128 * i : 128 * (i + 1), 0:512]
                    )
                for i in range(6):
                    nc.gpsimd.dma_start(
                        out=xt[i][:, 512:1024],
                        in_=xT_d[128 * i : 128 * (i + 1), 512:1024],
                    )
                for i in range(6):
                    nc.gpsimd.dma_start(
                        out=wq[i][:, 2 * D : E3],
                        in_=wqkvT_d[128 * i : 128 * (i + 1), 2 * D : E3],
                    )
                for i in range(6):
                    src, dst = _qk2(i)
                    nc.sync.dma_start(out=dst[:, :, 128:768], in_=src[:, :, 128:768])
                for i in range(6):
                    nc.sync.dma_start(
                        out=wp[i], in_=wprojT_d[128 * i : 128 * (i + 1), :]
                    )
            else:
                # contiguous chunks in consumption order; "sp" = SP-only
                # (v0 scheme), "sp_act" = x^T on ACT's HWDGE queue, q/k/v/wp
                # chunks on SP
                xt_eng = nc.scalar if dma_plan == "sp_act" else nc.sync

                def _xt_chunk(i, lo, hi):
                    xt_eng.dma_start(
                        out=xt[i][:, lo:hi], in_=xT_d[128 * i : 128 * (i + 1), lo:hi]
                    )

                def _wq_chunk(i, lo, hi):
                    nc.sync.dma_start(
                        out=wq[i][:, lo:hi],
                        in_=wqkvT_d[128 * i : 128 * (i + 1), lo:hi],
                    )

                if dma_plan == "sp_act":
                    for i in range(6):
                        _xt_chunk(i, 0, 512)
                    for i in range(6):
                        _xt_chunk(i, 512, 1024)
                    for i in range(6):
                        _wq_chunk(i, 0, 384)
                    for i in range(6):
                        _wq_chunk(i, 768, 1152)
                    for i in range(6):
                        _wq_chunk(i, 384, 768)
                    for i in range(6):
                        _wq_chunk(i, 1152, 1536)
                    for i in range(6):
                        _wq_chunk(i, 1536, 2304)
                else:
                    for i in range(6):
                        _xt_chunk(i, 0, 512)
                        _wq_chunk(i, 0, 384)
                    for i in range(6):
                        _xt_chunk(i, 512, 1024)
                        _wq_chunk(i, 768, 1152)
                    for i in range(6):
                        _wq_chunk(i, 1536, 1920)
                    for i in range(6):
                        _wq_chunk(i, 1920, 2304)
                    for i in range(6):
                        _wq_chunk(i, 384, 768)
                    for i in range(6):
                        _wq_chunk(i, 1152, 1536)
                for i in range(6):
                    nc.sync.dma_start(
                        out=wp[i], in_=wprojT_d[128 * i : 128 * (i + 1), :]
                    )

            # ---- PE clock warm-up ----
            # The HAM clock gate runs the PE at half rate until ~3.4us of
            # sustained activity. Burn the input-DMA wait on throwaway
            # matmuls so the first real matmuls run at full clock.
            for w in range(N_WARM):
                wps = ps_x.tile([128, 512], F32, tag="px", name="pswarm")
                nc.tensor.matmul(
                    wps, warm_sb[:, 0:128], warm_sb, start=True, stop=True
                )

            # ---- bias broadcast tiles (PE ones-matmul; replaces per-stile
            # bias matmuls). Doubles as extra warm-up. ----
            if with_bias:
                vb_bc = sb.tile([128, D], BF16, tag="vbbc")
                pb_bc = sb.tile([128, D], BF16, tag="pbbc")
                for half in range(2):
                    cols = slice(384 * half, 384 * half + 384)
                    ps = ps_mm.tile([128, 384], F32, tag="mm", name="psbc")
                    nc.tensor.matmul(
                        ps, ones_row[:, 0:128], vb_row[:, cols], start=True, stop=True
                    )
                    nc.vector.tensor_copy(out=vb_bc[:, cols], in_=ps)
                    ps2 = ps_mm.tile([128, 384], F32, tag="mm", name="psbc2")
                    nc.tensor.matmul(
                        ps2, ones_row[:, 0:128], pb_row[:, cols], start=True, stop=True
                    )
                    nc.vector.tensor_copy(out=pb_bc[:, cols], in_=ps2)

            # q/k stored as half-tiles so scores can start as soon as the
            # producing QKV half-group drains (finer Tile dep granularity)
            qkTh = [
                [
                    sb.tile([128, 512], BF16, tag=f"qk{i}h{f}", name=f"qk{i}h{f}")
                    for f in range(2)
                ]
                for i in range(12)
            ]
            v_aug = [
                sb.tile([128, H, DH + 1], BF16, tag=f"va{i}", name=f"va{i}")
                for i in range(8)
            ]
            projin = [
                sb.tile([128, S], BF16, tag=f"pj{i}", name=f"pj{i}") for i in range(6)
            ]
            if proj_split:
                ypart = [
                    sb.tile([128, D], F32R, tag=f"yp{i}", name=f"yp{i}")
                    for i in range(8)
                ]

            def qk_etile(et):
                # q/k column block [e, s] with per-e bias, accumulated over d
                for hf in range(2):
                    ps = ps_mm.tile([128, 512], F32, tag="mm", name="psqk")
                    for ki in range(6):
                        nc.tensor.matmul(
                            ps,
                            wq[ki][:, 128 * et : 128 * et + 128],
                            xt[ki][:, 512 * hf : 512 * hf + 512],
                            start=(ki == 0),
                            stop=(ki == 5),
                        )
                    if with_bias:
                        nc.vector.tensor_scalar_add(
                            qkTh[et][hf], ps, qb_col[:, et : et + 1]
                        )
                    else:
                        nc.vector.tensor_copy(out=qkTh[et][hf], in_=ps)

            def v_stile(st):
                nc.vector.memset(v_aug[st][:, :, DH : DH + 1], 1.0)
                for hf in range(2):
                    ps = ps_mm.tile([128, 384], F32, tag="mm", name="psv")
                    for ki in range(6):
                        nc.tensor.matmul(
                            ps,
                            xt[ki][:, 128 * st : 128 * st + 128],
                            wq[ki][:, 2 * D + 384 * hf : 2 * D + 384 * hf + 384],
                            start=(ki == 0),
                            stop=(ki == 5),
                        )
                    dst = v_aug[st][:, 6 * hf : 6 * hf + 6, 0:DH]
                    src = ps.rearrange("p (h d) -> p h d", h=6)
                    if with_bias:
                        nc.vector.tensor_tensor(
                            out=dst,
                            in0=src,
                            in1=vb_bc[:, 384 * hf : 384 * hf + 384].rearrange(
                                "p (h d) -> p h d", h=6
                            ),
                            op=mybir.AluOpType.add,
                        )
                    else:
                        nc.vector.tensor_copy(out=dst, in_=src)

            def head_scores(h):
                row = 64 * (h % 2)

                at = []
                for sk in range(8):
                    k_half = qkTh[6 + h // 2][sk // 4]
                    k_blk = k_half[row : row + 64, 128 * (sk % 4) : 128 * (sk % 4) + 128]
                    ps = ps_sc.tile([128, S], F32, tag="sc", name="pssc")
                    for hf in range(2):
                        nc.tensor.matmul(
                            ps[:, 512 * hf : 512 * hf + 512],
                            k_blk,
                            qkTh[h // 2][hf][row : row + 64, :],
                            start=True,
                            stop=True,
                        )
                    a = sb.tile([128, S], BF16, tag="at", bufs=32, name="at")
                    nc.scalar.activation(
                        out=a,
                        in_=ps,
                        func=mybir.ActivationFunctionType.Exp,
                        scale=SCALE,
                    )
                    at.append(a)
                return at

            def head_av_nat(h, at, g):
                # AV with attn^T stationary: out natural [sq, 65], M=128 rate.
                # 4 sq-blocks batched per PSUM bank to amortize engine hops;
                # normalize per-partition (sq). Returns the scaled tile for
                # the deferred transpose pass.
                nat = ps_x.tile([128, 4 * (DH + 1)], F32, tag="px", name="psnat")
                nat_r = nat.rearrange("p (j c) -> p j c", c=DH + 1)
                for sk in range(8):
                    for j in range(4):
                        nc.tensor.matmul(
                            nat_r[:, j, :],
                            at[sk][:, 128 * (4 * g + j) : 128 * (4 * g + j) + 128],
                            v_aug[sk][:, h, :],
                            start=(sk == 0 and j == 0),
                            stop=(sk == 7 and j == 3),
                        )
                rec = sb.tile([128, 4], F32, tag="rec", bufs=3, name="rec")
                nc.vector.reciprocal(out=rec, in_=nat_r[:, :, DH])
                scaled = sb.tile([128, 4 * DH], BF16, tag="scaled", bufs=3, name="scaled")
                rec_b = bass.AP(
                    tensor=rec.tensor,
                    offset=rec.offset,
                    ap=[rec.ap[0], rec.ap[1], [0, DH]],
                )
                nc.vector.tensor_mul(
                    scaled.rearrange("p (j d) -> p j d", j=4),
                    nat_r[:, :, 0:DH],
                    rec_b,
                )
                return scaled

            def head_av_tp(h, g, scaled):
                # PE-transpose the normalized AV block back to [d, sq]
                for t in range(2):
                    tp = ps_x.tile([128, 128], BF16, tag="px", name="pstp")
                    nc.tensor.transpose(
                        tp, scaled[:, 128 * t : 128 * t + 128], identity
                    )
                    for u in range(2):
                        sq = 4 * g + 2 * t + u
                        nc.vector.tensor_copy(
                            out=projin[h // 2][
                                64 * (h % 2) : 64 * (h % 2) + 64,
                                128 * sq : 128 * sq + 128,
                            ],
                            in_=tp[64 * u : 64 * u + 64, :],
                        )

            def head_av(h, at):
                for g in range(2):
                    scaled = head_av_nat(h, at, g)
                    head_av_tp(h, g, scaled)

            def proj_hf(st, kis, hf, first, last, split_pools, y):
                # one hf-half ki-range partial of the projection for block st.
                # Non-first groups fold the f32 partial back in on the PE via
                # an f32r identity matmul (1 cycle/row), so the drain is a
                # plain PSUM->SBUF copy that ACT and DVE split between them.
                pl = ps_sc if (split_pools and hf == 1) else ps_mm
                ps = pl.tile([128, 384], F32, tag="mm" if pl is ps_mm else "sc", name="psy")
                for n, ki in enumerate(kis):
                    nc.tensor.matmul(
                        ps,
                        projin[ki][:, 128 * st : 128 * st + 128],
                        wp[ki][:, 384 * hf : 384 * hf + 384],
                        start=(n == 0),
                        stop=(first and n == len(kis) - 1),
                    )
                cols = slice(384 * hf, 384 * hf + 384)
                if not first:
                    nc.tensor.matmul(
                        ps,
                        ident32,
                        ypart[st][:, cols],
                        start=False,
                        stop=True,
                    )
                if last:
                    if with_bias and first:
                        nc.vector.tensor_tensor(
                            out=y[:, cols], in0=ps, in1=pb_bc[:, cols],
                            op=mybir.AluOpType.add,
                        )
                    elif hf == 0:
                        nc.scalar.activation(
                            out=y[:, cols], in_=ps,
                            func=mybir.ActivationFunctionType.Copy,
                        )
                    else:
                        nc.vector.tensor_copy(out=y[:, cols], in_=ps)
                else:
                    if with_bias:
                        nc.vector.tensor_tensor(
                            out=ypart[st][:, cols], in0=ps, in1=pb_bc[:, cols],
                            op=mybir.AluOpType.add,
                        )
                    else:
                        nc.vector.tensor_copy(out=ypart[st][:, cols], in_=ps)

            def proj_group(st, kis, first, last, split_pools=False, dma_eng=None):
                y = (
                    sb.tile([128, D], BF16, tag="y", bufs=3, name="y")
                    if last
                    else None
                )
                for hf in range(2):
                    proj_hf(st, kis, hf, first, last, split_pools, y)
                if last:
                    eng = dma_eng if dma_eng is not None else nc.sync
                    eng.dma_start(out=out_d[128 * st : 128 * st + 128, :], in_=y)

            # ---- interleaved schedule ----
            ats = {}
            pend = []
            for p in range(6):
                qk_etile(p)
                qk_etile(6 + p)
                for h in (2 * p, 2 * p + 1):
                    if h in V_AT:
                        for st in range(*V_AT[h]):
                            v_stile(st)
                    ats[h] = head_scores(h)
                    pend.append(h)
                    if len(pend) > AV_LAG:
                        hh = pend.pop(0)
                        head_av(hh, ats.pop(hh))

            # tail phase: scores psum pool is free from here — alternate proj
            # groups across ps_mm/ps_sc so PSUM drains overlap, and weave the
            # remaining AVs between proj pairs to cover drain latency.
            if proj_split:
                # AV nat groups lead (their exps are long done); each proj-A
                # group covers the normalize->transpose latency of the AV
                # half it is woven between. proj-B + out-DMA stream last so
                # the DMA engines start flowing as early as possible.
                avq = [(h, g) for h in pend for g in range(2)]
                for st in range(8):
                    h, g = avq[st]
                    scaled = head_av_nat(h, ats[h], g)
                    proj_hf(st, range(4), 0, True, False, True, None)
                    head_av_tp(h, g, scaled)
                    proj_hf(st, range(4), 1, True, False, True, None)
                for hh in pend:
                    ats.pop(hh)
                for st in range(8):
                    proj_group(
                        st, range(4, 6), first=False, last=True,
                        split_pools=True,
                        dma_eng=nc.sync if st % 2 == 0 else nc.scalar,
                    )
            else:
                for hh in pend:
                    head_av(hh, ats.pop(hh))
                for st in range(8):
                    proj_group(
                        st, range(6), first=True, last=True,
                        split_pools=True,
                        dma_eng=nc.sync if st % 2 == 0 else nc.scalar,
                    )

    if do_compile:
        nc.compile()
    return nc


_NCS = {}


def _get_nc(with_bias=True):
    if with_bias not in _NCS:
        _NCS[with_bias] = build_nc(with_bias=with_bias)
    return _NCS[with_bias]


def make_in_maps(x, qkv_w, qkv_b, proj_w, proj_b):
    x = np.asarray(x, dtype=np.float32)
    qkv_w = np.asarray(qkv_w, dtype=np.float32)
    qkv_b = np.asarray(qkv_b, dtype=np.float32)
    proj_w = np.asarray(proj_w, dtype=np.float32)
    proj_b = np.asarray(proj_b, dtype=np.float32)

    xT = np.ascontiguousarray(x.transpose(0, 2, 1)).astype(ml_dtypes.bfloat16)
    wqkvT = np.ascontiguousarray(qkv_w.T).astype(ml_dtypes.bfloat16)
    wprojT = np.ascontiguousarray(proj_w.T).astype(ml_dtypes.bfloat16)
    return [
        {
            "xT": xT[b],
            "wqkvT": wqkvT,
            "wprojT": wprojT,
            "qkvb": qkv_b,
            "projb": proj_b,
        }
        for b in range(N_CORES)
    ]


def kernel(x, qkv_w, qkv_b, proj_w, proj_b):
    from concourse.bass_utils import run_bass_kernel_spmd

    in_maps = make_in_maps(x, qkv_w, qkv_b, proj_w, proj_b)
    with_bias = bool(np.any(np.asarray(qkv_b)) or np.any(np.asarray(proj_b)))
    nc = _get_nc(with_bias)
    res = run_bass_kernel_spmd(nc, in_maps, core_ids=list(range(N_CORES))).results
    return np.stack([res[b]["out"] for b in range(N_CORES)]).astype(np.float32)
